# revision 2
# baseline (speedup 1.0000x reference)
"""Cox proportional-hazards loss on 8 Trainium2 NeuronCores.

Math (reference):
    order = argsort(-times, stable)
    s = log_risks[order]; m = censor[order]
    c_i = cumsum(exp(s))_i                      (global, over sorted order)
    loss = -(sum_i m_i*s_i - sum_i m_i*log(c_i)) / max(sum_i m_i, 1)

Strategy:
  - Host: stable sort by descending time (sharding hint allows host pre-sort),
    shard the sorted arrays contiguously across 8 cores, each core's shard laid
    out [128 partitions, F] with each partition holding a contiguous run.
  - Device, per core:
      e = exp(s)                      ScalarE (accum_out -> per-partition sums)
      c_local = scan(e) per partition VectorE tensor_tensor_scan (fp32 state)
      partition offsets               TensorE: strict-lower-triangular matmul
      core offset                     AllGather of per-core totals + masked dot
      v = (c_local + B)*m             fused scalar_tensor_tensor,
                                      B[p] = part_off[p] + core_off - 1
      masked log sum                  ScalarE Ln(v + 1) with accum_out
                                      (m=0 -> ln(1)=0, m=1 -> ln(c_global))
      masked s sum, event count       fused DVE ops with accum_out
  - Each core writes [128, 3] partials (sum m*s, sum m*log c, sum m);
    host reduces 8x128x3 and forms the scalar (the unshard step).
"""

import sys

sys.path.insert(0, "/opt/trn_rl_repo")

import numpy as np

import concourse.bass as bass
import concourse.bacc as bacc
import concourse.tile as tile
from concourse import mybir
from concourse import bass_utils

N = 8388608
NCORES = 8
P = 128
F = N // (NCORES * P)  # 8192
K = 4  # free-dim chunks per core

FP32 = mybir.dt.float32
BF16 = mybir.dt.bfloat16
BF16_NP = mybir.dt.np(BF16)


def build(F=F, K=K, ncores=NCORES, debug=False):
    """Build the SPMD program. Returns the Bacc object (compiled)."""
    Fc = F // K
    nc = bacc.Bacc(
        "TRN2", target_bir_lowering=False, debug=debug, num_devices=ncores
    )

    slr_d = nc.dram_tensor("slr", [P, F], BF16, kind="ExternalInput")
    msk_d = nc.dram_tensor("msk", [P, F], BF16, kind="ExternalInput")
    mvec_d = nc.dram_tensor("mvec", [1, ncores], FP32, kind="ExternalInput")
    ltri_d = nc.dram_tensor("ltri", [P, P], FP32, kind="ExternalInput")
    out_d = nc.dram_tensor("out", [P, 3], FP32, kind="ExternalOutput")

    with tile.TileContext(nc) as tc:
        with (
            tc.tile_pool(name="resident", bufs=1) as res,
            tc.tile_pool(name="slr_chunks", bufs=3) as slr_pool,
            tc.tile_pool(name="e_chunks", bufs=3) as e_pool,
            tc.tile_pool(name="v_chunks", bufs=2) as v_pool,
            tc.tile_pool(name="w_chunks", bufs=2) as w_pool,
            tc.tile_pool(name="scr_chunks", bufs=2) as scr_pool,
            tc.tile_pool(name="psum", bufs=1, space="PSUM") as psum,
            tc.tile_pool(name="dram", bufs=1, space="DRAM") as dram,
        ):
            # Resident tiles
            c_full = res.tile([P, F], FP32)      # local inclusive cumsum
            m_full = res.tile([P, F], BF16)      # event mask
            rowsumE = res.tile([P, K], FP32)     # per-chunk exp row sums
            msl = res.tile([P, K], FP32)         # per-chunk masked s sums
            mlog = res.tile([P, K], FP32)        # per-chunk masked log sums
            cnt = res.tile([P, K], FP32)         # per-chunk event counts
            onesF = res.tile([P, Fc], BF16)      # scan data0
            ones_col = res.tile([P, 1], FP32)
            ones_row = res.tile([1, P], FP32)
            ltri = res.tile([P, P], FP32)
            mvec = res.tile([1, ncores], FP32)
            allT = res.tile([1, ncores], FP32)
            junk8 = res.tile([1, ncores], FP32)
            coreoff = res.tile([1, 1], FP32)
            coreoffm1 = res.tile([1, 1], FP32)
            rowtot = res.tile([P, 1], FP32)
            ct_sb = res.tile([1, 1], FP32)
            b_sb = res.tile([P, 1], FP32)
            stats = res.tile([P, 3], FP32)

            nc.gpsimd.memset(onesF[:], 1.0)
            nc.gpsimd.memset(ones_col[:], 1.0)
            nc.gpsimd.memset(ones_row[:], 1.0)
            nc.sync.dma_start(ltri[:], ltri_d[:, :])
            nc.sync.dma_start(mvec[:], mvec_d[:, :])

            # ---- Phase 1: per chunk: load, exp, scan, masked-s, count ----
            for k in range(K):
                ck = bass.ts(k, Fc)
                slr_k = slr_pool.tile([P, Fc], BF16, name=f"slr_{k}", tag="slr")
                nc.sync.dma_start(slr_k[:], slr_d[:, ck])
                nc.sync.dma_start(m_full[:, ck], msk_d[:, ck])

                e_k = e_pool.tile([P, Fc], BF16, name=f"e_{k}", tag="e")
                nc.scalar.activation(
                    e_k[:],
                    slr_k[:],
                    mybir.ActivationFunctionType.Exp,
                    accum_out=rowsumE[:, k : k + 1],
                )

                init = 0.0 if k == 0 else c_full[:, k * Fc - 1 : k * Fc]
                nc.vector.tensor_tensor_scan(
                    c_full[:, ck],
                    onesF[:],
                    e_k[:],
                    init,
                    op0=mybir.AluOpType.mult,
                    op1=mybir.AluOpType.add,
                )

                scr_k = scr_pool.tile([P, Fc], BF16, name=f"scr_{k}", tag="scr")
                nc.vector.scalar_tensor_tensor(
                    scr_k[:],
                    slr_k[:],
                    1.0,
                    m_full[:, ck],
                    op0=mybir.AluOpType.mult,
                    op1=mybir.AluOpType.mult,
                    accum_out=msl[:, k : k + 1],
                )

                scr2_k = scr_pool.tile([P, Fc], BF16, name=f"scr2_{k}", tag="scr2")
                # op1 is the accum reduce op when accum_out is set
                nc.vector.tensor_scalar(
                    scr2_k[:],
                    m_full[:, ck],
                    1.0,
                    None,
                    op0=mybir.AluOpType.mult,
                    op1=mybir.AluOpType.add,
                    accum_out=cnt[:, k : k + 1],
                )

            # ---- Phase 1b: offsets ----
            nc.vector.tensor_reduce(
                rowtot[:], rowsumE[:], mybir.AxisListType.X, mybir.AluOpType.add
            )
            # partition offsets: po[q] = sum_{p<q} rowtot[p]
            po_ps = psum.tile([P, 1], FP32)
            nc.tensor.matmul(po_ps[:], ltri[:], rowtot[:], start=True, stop=False)
            # core total -> DRAM -> AllGather
            ct_ps = psum.tile([1, 1], FP32)
            nc.tensor.matmul(ct_ps[:], rowtot[:], ones_col[:], start=True, stop=True)
            nc.scalar.copy(ct_sb[:], ct_ps[:])
            cc_in = dram.tile([1, 1], FP32)
            cc_out = dram.tile([ncores, 1], FP32, addr_space="Shared")
            nc.sync.dma_start(cc_in[:], ct_sb[:])
            nc.gpsimd.collective_compute(
                "AllGather",
                mybir.AluOpType.bypass,
                replica_groups=[list(range(ncores))],
                ins=[cc_in[:].opt()],
                outs=[cc_out[:].opt()],
            )
            nc.sync.dma_start(allT[:], cc_out[:].rearrange("a b -> b a"))
            # core offset = sum_j<k allT[j]  (mvec is the per-core mask)
            nc.vector.scalar_tensor_tensor(
                junk8[:],
                allT[:],
                1.0,
                mvec[:],
                op0=mybir.AluOpType.mult,
                op1=mybir.AluOpType.mult,
                accum_out=coreoff[:],
            )
            nc.vector.tensor_scalar_add(coreoffm1[:], coreoff[:], -1.0)
            # broadcast (coreoff-1) to all partitions, accumulate onto po_ps
            nc.tensor.matmul(
                po_ps[:], ones_row[:], coreoffm1[:], start=False, stop=True
            )
            nc.scalar.copy(b_sb[:], po_ps[:])

            # ---- Phase 2: blend + masked log ----
            for k in range(K):
                ck = bass.ts(k, Fc)
                v_k = v_pool.tile([P, Fc], FP32, name=f"v_{k}", tag="v")
                nc.vector.scalar_tensor_tensor(
                    v_k[:],
                    c_full[:, ck],
                    b_sb[:],
                    m_full[:, ck],
                    op0=mybir.AluOpType.add,
                    op1=mybir.AluOpType.mult,
                )
                w_k = w_pool.tile([P, Fc], BF16, name=f"w_{k}", tag="w")
                nc.scalar.activation(
                    w_k[:],
                    v_k[:],
                    mybir.ActivationFunctionType.Ln,
                    bias=1.0,
                    accum_out=mlog[:, k : k + 1],
                )

            # ---- Phase 3: fold chunk stats and store ----
            nc.vector.tensor_reduce(
                stats[:, 0:1], msl[:], mybir.AxisListType.X, mybir.AluOpType.add
            )
            nc.vector.tensor_reduce(
                stats[:, 1:2], mlog[:], mybir.AxisListType.X, mybir.AluOpType.add
            )
            nc.vector.tensor_reduce(
                stats[:, 2:3], cnt[:], mybir.AxisListType.X, mybir.AluOpType.add
            )
            nc.sync.dma_start(out_d[:, :], stats[:])

    nc.compile()
    return nc


_NC_CACHE = {}


def _get_nc():
    key = (F, K, NCORES)
    if key not in _NC_CACHE:
        _NC_CACHE[key] = build()
    return _NC_CACHE[key]


def _make_in_maps(log_risks, times, censor, F=F, ncores=NCORES):
    order = np.argsort(-times, kind="stable")
    slr = np.ascontiguousarray(log_risks[order]).astype(BF16_NP)
    msk = censor[order].astype(BF16_NP)
    slr = slr.reshape(ncores, P, F)
    msk = msk.reshape(ncores, P, F)
    ltri = np.triu(np.ones((P, P), dtype=np.float32), k=1)
    in_maps = []
    for k in range(ncores):
        mvec = np.zeros((1, ncores), dtype=np.float32)
        mvec[0, :k] = 1.0
        in_maps.append(
            {"slr": slr[k], "msk": msk[k], "mvec": mvec, "ltri": ltri}
        )
    return in_maps


def _combine(results):
    msl = mlog = cnt = 0.0
    for r in results:
        o = r["out"].astype(np.float64)
        msl += o[:, 0].sum()
        mlog += o[:, 1].sum()
        cnt += o[:, 2].sum()
    if cnt <= 0:
        return np.float32(0.0)
    total = msl - mlog
    return np.float32(-total / cnt)


def run(log_risks, times, censor, trace=False):
    nc = _get_nc()
    in_maps = _make_in_maps(log_risks, times, censor)
    res = bass_utils.run_bass_kernel_spmd(
        nc, in_maps, core_ids=list(range(NCORES)), trace=trace
    )
    return _combine(res.results), res


def kernel(log_risks, times, censor):
    out, _ = run(log_risks, times, censor)
    return out


# revision 9
# speedup vs baseline: 1.8697x; 1.8697x over previous
"""Cox proportional-hazards loss on 8 Trainium2 NeuronCores.

Math (reference):
    order = argsort(-times, stable)
    s = log_risks[order]; m = censor[order]
    c_i = cumsum(exp(s))_i                      (global, over sorted order)
    loss = -(sum_i m_i*s_i - sum_i m_i*log(c_i)) / max(sum_i m_i, 1)

Strategy:
  - Host: stable sort by descending time (sharding hint allows host pre-sort),
    shard the sorted arrays contiguously across 8 cores, each core's shard laid
    out [128 partitions, F] with each partition holding a contiguous run.
  - Device, per core:
      e = exp(s)                      ScalarE (accum_out -> per-partition sums)
      c_local = scan(e) per partition VectorE tensor_tensor_scan (fp32 state)
      partition offsets               TensorE: strict-lower-triangular matmul
      core offset                     AllGather of per-core totals + masked dot
                                      (or host-computed exclusive shard prefix)
      v = (c_local + B)*m             fused scalar_tensor_tensor,
                                      B[p] = part_off[p] + core_off - 1
      masked log sum                  ScalarE Ln(v + 1) with accum_out
                                      (m=0 -> ln(1)=0, m=1 -> ln(c_global))
      masked s sum                    fused DVE scalar_tensor_tensor accum
      event count                     TensorE ones^T @ m column-sum matmuls
  - Each core writes [128, 3] partials (sum m*s, sum m*log c, sum m);
    host reduces 8x128x3 and forms the scalar (the unshard step).
"""

import sys

sys.path.insert(0, "/opt/trn_rl_repo")

import numpy as np

import concourse.bass as bass
import concourse.bacc as bacc
import concourse.tile as tile
from concourse import mybir
from concourse import bass_utils

N = 8388608
NCORES = 8
P = 128
F = N // (NCORES * P)  # 8192
K = 4                  # free-dim compute chunks per core
DMA_SPLIT = 2          # input DMA transfers per tensor

MODE = "collective"    # "collective" | "host" (cross-core exclusive prefix)
SCAN_F32 = True        # exp output / scan operand dtype

FP32 = mybir.dt.float32
BF16 = mybir.dt.bfloat16
BF16_NP = mybir.dt.np(BF16)


def build(F=F, K=K, ncores=NCORES, mode=MODE, scan_f32=SCAN_F32, debug=False):
    """Build the SPMD program. Returns the compiled Bacc object."""
    Fc = F // K
    E_DT = FP32 if scan_f32 else BF16
    nc = bacc.Bacc(
        "TRN2", target_bir_lowering=False, debug=debug, num_devices=ncores
    )

    slr_d = nc.dram_tensor("slr", [P, F], BF16, kind="ExternalInput")
    msk_d = nc.dram_tensor("msk", [P, F], BF16, kind="ExternalInput")
    if mode == "collective":
        mvec_d = nc.dram_tensor("mvec", [ncores + 1, 1], FP32, kind="ExternalInput")
        ltri_d = nc.dram_tensor("ltri", [P, P], FP32, kind="ExternalInput")
    else:
        boff_d = nc.dram_tensor("boff", [P, 1], FP32, kind="ExternalInput")
    out_d = nc.dram_tensor("out", [P, 3], FP32, kind="ExternalOutput")

    with tile.TileContext(nc) as tc:
        with (
            tc.tile_pool(name="resident", bufs=1) as res,
            tc.tile_pool(name="e_chunks", bufs=3) as e_pool,
            tc.tile_pool(name="v_chunks", bufs=2) as v_pool,
            tc.tile_pool(name="w_chunks", bufs=2) as w_pool,
            tc.tile_pool(name="scr_chunks", bufs=2) as scr_pool,
            tc.tile_pool(name="psum", bufs=1, space="PSUM") as psum,
            tc.tile_pool(name="dram", bufs=1, space="DRAM") as dram,
        ):
            # Resident tiles
            slr_full = res.tile([P, F], BF16)
            m_full = res.tile([P, F], BF16)
            c_full = res.tile([P, F], FP32)      # local inclusive cumsum
            rowsumE = res.tile([P, K], FP32)     # per-chunk exp row sums
            rowscr = res.tile([P, K], FP32)
            msl = res.tile([P, K], FP32)         # per-chunk masked s sums
            mlog = res.tile([P, K], FP32)        # per-chunk masked log sums
            onesF = res.tile([P, Fc], E_DT)      # scan data0
            ones_colb = res.tile([P, 1], BF16)   # count matmul lhsT
            ones_col = res.tile([P, 1], FP32)
            ones_row = res.tile([1, P], FP32)
            rowtot = res.tile([P, 1], FP32)
            ct_sb = res.tile([1, 1], FP32)
            b_sb = res.tile([P, 1], FP32)
            stats = res.tile([P, 3], FP32)

            nc.gpsimd.memset(onesF[:], 1.0)
            nc.gpsimd.memset(ones_colb[:], 1.0)
            nc.gpsimd.memset(ones_col[:], 1.0)
            nc.gpsimd.memset(ones_row[:], 1.0)
            nc.gpsimd.memset(stats[:], 0.0)

            if mode == "collective":
                ltri = res.tile([P, P], FP32)
                mvec = res.tile([ncores + 1, 1], FP32)
                allT = res.tile([ncores + 1, 1], FP32)
                coreoffm1 = res.tile([1, 1], FP32)
                nc.gpsimd.memset(allT[:], 1.0)  # row `ncores` stays 1.0
                nc.sync.dma_start(ltri[:], ltri_d[:, :])
                nc.sync.dma_start(mvec[:], mvec_d[:, :])

            # ---- Input loads (big transfers) ----
            Fd = F // DMA_SPLIT
            for j in range(DMA_SPLIT):
                cj = bass.ts(j, Fd)
                nc.sync.dma_start(slr_full[:, cj], slr_d[:, cj])
                nc.sync.dma_start(m_full[:, cj], msk_d[:, cj])

            # ---- Phase 1a: exp with row-sum accum (ScalarE) ----
            e_ks = []
            for k in range(K):
                ck = bass.ts(k, Fc)
                e_k = e_pool.tile([P, Fc], E_DT, name=f"e_{k}", tag="e")
                e_ks.append(e_k)
                nc.scalar.activation(
                    e_k[:],
                    slr_full[:, ck],
                    mybir.ActivationFunctionType.Exp,
                    accum_out=rowsumE[:, k : k + 1],
                )

            # ---- Phase 1b per chunk: scan, masked-s, count ----
            mm_w = min(512, Fc)
            nmm = Fc // mm_w
            cnt_ps = psum.tile([1, mm_w], FP32)
            for k in range(K):
                ck = bass.ts(k, Fc)
                init = 0.0 if k == 0 else c_full[:, k * Fc - 1 : k * Fc]
                nc.vector.tensor_tensor_scan(
                    c_full[:, ck],
                    onesF[:],
                    e_ks[k][:],
                    init,
                    op0=mybir.AluOpType.mult,
                    op1=mybir.AluOpType.add,
                )

                scr_k = scr_pool.tile([P, Fc], BF16, name=f"scr_{k}", tag="scr")
                nc.vector.scalar_tensor_tensor(
                    scr_k[:],
                    slr_full[:, ck],
                    1.0,
                    m_full[:, ck],
                    op0=mybir.AluOpType.mult,
                    op1=mybir.AluOpType.mult,
                    accum_out=msl[:, k : k + 1],
                )

                # event count: PE column sums accumulated into one PSUM strip
                for j in range(nmm):
                    nc.tensor.matmul(
                        cnt_ps[:],
                        ones_colb[:],
                        m_full[:, k * Fc + j * mm_w : k * Fc + (j + 1) * mm_w],
                        start=(k == 0 and j == 0),
                        stop=(k == K - 1 and j == nmm - 1),
                    )

            # ---- Offset chain (no VectorE ops; DVE must not stall) ----
            # rowtot = sum_k rowsumE[:, k]  on ScalarE via Identity+accum
            nc.scalar.activation(
                rowscr[:],
                rowsumE[:],
                mybir.ActivationFunctionType.Identity,
                accum_out=rowtot[:],
            )
            po_ps = psum.tile([P, 1], FP32)
            if mode == "collective":
                # partition offsets: po[q] = sum_{p<q} rowtot[p]
                nc.tensor.matmul(po_ps[:], ltri[:], rowtot[:], start=True, stop=False)
                # core total -> DRAM -> AllGather
                ct_ps = psum.tile([1, 1], FP32)
                nc.tensor.matmul(
                    ct_ps[:], rowtot[:], ones_col[:], start=True, stop=True
                )
                nc.scalar.copy(ct_sb[:], ct_ps[:])
                cc_in = dram.tile([1, 1], FP32)
                cc_out = dram.tile([ncores, 1], FP32, addr_space="Shared")
                nc.sync.dma_start(cc_in[:], ct_sb[:])
                nc.gpsimd.collective_compute(
                    "AllGather",
                    mybir.AluOpType.bypass,
                    replica_groups=[list(range(ncores))],
                    ins=[cc_in[:].opt()],
                    outs=[cc_out[:].opt()],
                )
                nc.sync.dma_start(allT[0:ncores, :], cc_out[:, :])
                # coreoff-1 = sum_{j<rank} allT[j] - 1 via PE dot (contract
                # over partitions; mvec row ncores = -1, allT row ncores = 1);
                # no VectorE/GpSimd ops so their queues never wait on the
                # collective
                co_ps = psum.tile([1, 1], FP32)
                nc.tensor.matmul(co_ps[:], allT[:], mvec[:], start=True, stop=True)
                nc.scalar.copy(coreoffm1[:], co_ps[:])
                # broadcast (coreoff-1) to all partitions, accumulate onto po_ps
                nc.tensor.matmul(
                    po_ps[:], ones_row[:], coreoffm1[:], start=False, stop=True
                )
                nc.scalar.copy(b_sb[:], po_ps[:])
            else:
                # host supplies exclusive prefix of exp sums for this core's
                # partition rows, minus 1 (the global shard prefix folded in)
                nc.sync.dma_start(b_sb[:], boff_d[:, :])

            # ---- Phase 2 per chunk: blend + masked log ----
            for k in range(K):
                ck = bass.ts(k, Fc)
                v_k = v_pool.tile([P, Fc], FP32, name=f"v_{k}", tag="v")
                nc.vector.scalar_tensor_tensor(
                    v_k[:],
                    c_full[:, ck],
                    b_sb[:],
                    m_full[:, ck],
                    op0=mybir.AluOpType.add,
                    op1=mybir.AluOpType.mult,
                )
                w_k = w_pool.tile([P, Fc], BF16, name=f"w_{k}", tag="w")
                nc.scalar.activation(
                    w_k[:],
                    v_k[:],
                    mybir.ActivationFunctionType.Ln,
                    bias=1.0,
                    accum_out=mlog[:, k : k + 1],
                )

            # ---- Phase 3: fold chunk stats and store ----
            nc.vector.tensor_reduce(
                stats[:, 0:1], msl[:], mybir.AxisListType.X, mybir.AluOpType.add
            )
            nc.vector.tensor_reduce(
                stats[:, 1:2], mlog[:], mybir.AxisListType.X, mybir.AluOpType.add
            )
            nc.vector.tensor_reduce(
                stats[0:1, 2:3], cnt_ps[:], mybir.AxisListType.X, mybir.AluOpType.add
            )
            nc.sync.dma_start(out_d[:, :], stats[:])

    nc.compile()
    return nc


_NC_CACHE = {}


def _get_nc(mode=MODE, scan_f32=SCAN_F32):
    key = (F, K, NCORES, mode, scan_f32)
    if key not in _NC_CACHE:
        _NC_CACHE[key] = build(mode=mode, scan_f32=scan_f32)
    return _NC_CACHE[key]


def _make_in_maps(log_risks, times, censor, mode=MODE, F=F, ncores=NCORES):
    order = np.argsort(-times, kind="stable")
    slr = np.ascontiguousarray(log_risks[order]).astype(BF16_NP)
    msk = censor[order].astype(BF16_NP)
    slr = slr.reshape(ncores, P, F)
    msk = msk.reshape(ncores, P, F)
    in_maps = []
    if mode == "collective":
        ltri = np.triu(np.ones((P, P), dtype=np.float32), k=1)
        for k in range(ncores):
            mvec = np.zeros((ncores + 1, 1), dtype=np.float32)
            mvec[:k, 0] = 1.0
            mvec[ncores, 0] = -1.0
            in_maps.append(
                {"slr": slr[k], "msk": msk[k], "mvec": mvec, "ltri": ltri}
            )
    else:
        # exclusive prefix of per-partition-row exp sums across the whole
        # sorted array (host-side "exclusive prefix of per-shard totals")
        rows = np.exp(slr.astype(np.float64)).sum(axis=2).reshape(-1)
        pref = np.concatenate([[0.0], np.cumsum(rows)[:-1]])
        pref = pref.reshape(ncores, P, 1).astype(np.float32) - 1.0
        for k in range(ncores):
            in_maps.append({"slr": slr[k], "msk": msk[k], "boff": pref[k]})
    return in_maps


def _combine(results):
    msl = mlog = cnt = 0.0
    for r in results:
        o = r["out"].astype(np.float64)
        msl += o[:, 0].sum()
        mlog += o[:, 1].sum()
        cnt += o[:, 2].sum()
    if cnt <= 0:
        return np.float32(0.0)
    total = msl - mlog
    return np.float32(-total / cnt)


def run(log_risks, times, censor, trace=False, mode=MODE, scan_f32=SCAN_F32):
    nc = _get_nc(mode, scan_f32)
    in_maps = _make_in_maps(log_risks, times, censor, mode=mode)
    res = bass_utils.run_bass_kernel_spmd(
        nc, in_maps, core_ids=list(range(NCORES)), trace=trace
    )
    return _combine(res.results), res


def kernel(log_risks, times, censor):
    out, _ = run(log_risks, times, censor)
    return out
